# revision 1
# baseline (speedup 1.0000x reference)
"""Conv1d (K=5, pad=2) with folded LoRA on 8 Trainium2 NeuronCores.

Strategy
--------
Data-parallel: batch 8 -> 1 batch item per core. The LoRA path is folded
into the conv weights on the host (exact up to fp32 rounding):
    W_eff = conv_w + (alpha/rank) * einsum('or,rik->oik', lora_B, lora_A)
so the device kernel is a single conv1d + bias.

Per core: y[co, t] = bias[co] + sum_{k,ci} W_eff[co, ci, k] * x[ci, t+k-2]
computed as 5 shifted matmuls accumulating in PSUM, over 2 ci-blocks and
2 co-blocks of 128, in fp32r (TF32-class PE mode, 1 cycle/row; ~1.5e-4
scale-relative absmax at K=128 contraction, measured on HW).

Toolchain constraint baked into the structure: every instruction may carry
at most ONE sync wait (walrus setupSyncWait limit), and Tile's wait elision
is per-proc (engine vs sequencer are distinct procs, no transitivity).
Hence:
  - PE "observer" matmuls (1-column, scratch PSUM) absorb each x/weight DMA
    lane wait so real matmuls only wait on the DVE sem (PSUM-bank WAR).
  - Evictions (PSUM->SBUF + bias add) run exclusively on DVE and wait only
    on PE; out-DMA-slot WAR is absorbed by tiny DVE memsets; the bias lane
    by a tiny DVE copy.
  - x-loads ride the SP HWDGE ring, stores the ACT HWDGE ring; same-ring
    WAW lane waits are absorbed by sequencer nops on the matching ring.
  - A tail chain of 1-dep sync nops covers all procs so the exit drain
    carries at most one wait.
"""
import sys
sys.path.insert(0, "/opt/trn_rl_repo")
import numpy as np

from concourse import bass, mybir, tile
from concourse import bass_utils
from concourse.tile import add_dep_helper

# Problem constants (hardcoded per contract)
B = 8
CI = 256
CO = 256
K = 5
PAD = 2
T = 16384
RANK = 8
ALPHA = 16.0
SCALING = ALPHA / RANK
N_CORES = 8

# Tiling
CHUNK = 1024          # output columns per chunk
NCHUNK = T // CHUNK   # 16
SUB = 512             # matmul free dim
NSUB = CHUNK // SUB   # 2
XCOLS = CHUNK + 2 * PAD  # chunk + halo


def _build_nc(reps=1):
    f32 = mybir.dt.float32
    f32r = mybir.dt.float32r

    nc = bass.Bass(trn_type="TRN2", debug=False)
    x = nc.dram_tensor("x", [CI, T], f32, kind="ExternalInput").ap()
    wts = nc.dram_tensor("wts", [128, K * 2 * 2 * 128], f32, kind="ExternalInput").ap()
    bias = nc.dram_tensor("bias", [128, 2], f32, kind="ExternalInput").ap()
    zeros = nc.dram_tensor("zeros", [128, 2, PAD], f32, kind="ExternalInput").ap()
    # one output tensor per 2048-wide super-chunk, stored via SWDGE so each
    # store owns a DMASW lane exactly once (no lane-predecessor wait); host
    # concatenates
    ys = [nc.dram_tensor(f"y{s}", [CI, 2 * CHUNK], f32, kind="ExternalOutput").ap()
          for s in range(NCHUNK // 2)]

    xab = x.rearrange("(b p) t -> p b t", p=128)
    ysab = [yc.rearrange("(b p) t -> p b t", p=128) for yc in ys]

    NOB = 2   # out staging buffers (super-chunks)
    NPB = 6   # psum accumulation banks

    with tile.TileContext(nc) as tc:
        with tc.tile_pool(name="wp", bufs=1) as wp, \
             tc.tile_pool(name="pp", bufs=1, space="PSUM") as pp:

            # write-once observer scratch: two columns per observer matmul
            # (fp32r APs need 8-byte alignment)
            obs_ps = pp.tile([128, 64], f32, name="obs_ps", tag="obs")
            pbufs = [pp.tile([128, SUB], f32, name=f"pt{j}", tag=f"pt{j}")
                     for j in range(NPB)]
            # x is fully resident: one dedicated buffer per chunk, no reuse
            xbufs = [wp.tile([128, 2, XCOLS], f32r, name=f"xt{j}", tag=f"xt{j}")
                     for j in range(NCHUNK)]
            obufs = [wp.tile([128, 2, 2 * CHUNK], f32, name=f"ot{j}", tag=f"ot{j}")
                     for j in range(NOB)]
            # write-once DVE gate scratch: one column per gate memset
            gs = wp.tile([128, 4 * NCHUNK * reps + 8], f32, name="gs")

            wr = wp.tile([128, K * 2 * 2 * 128], f32r, name="wr")
            d_w = nc.sync.dma_start(out=wr[:], in_=wts[:].bitcast(f32r))
            bs = wp.tile([128, 2], f32, name="bs")
            d_b = nc.sync.dma_start(out=bs[:], in_=bias[:])

            n_obs = [0]

            def pe_observe(src_ap, dma_inst):
                """1-column matmul whose only wait is `dma_inst`'s lane.

                Reads only within the region `dma_inst` wrote; writes its own
                never-reused obs_ps column (no WAW chain)."""
                n = src_ap.shape[-1]
                m = min(128, n)
                oc = 2 * n_obs[0]
                n_obs[0] += 1
                mm = nc.tensor.matmul(obs_ps[0:m, oc:oc + 2], src_ap[:, 0:m],
                                      src_ap[:, 0:2], start=True, stop=True)
                add_dep_helper(mm.ins, dma_inst.ins, sync=False, reason="obs-order")
                return mm

            n_gate = [0]

            def dve_gate(dep_inst):
                """Write-once DVE memset whose only wait is dep's proc tick."""
                gc = n_gate[0]
                n_gate[0] += 1
                ms = nc.vector.memset(gs[:, gc:gc + 1], 0.0)
                add_dep_helper(ms.ins, dep_inst.ins, sync=True, reason="dve-gate")
                return ms

            obs_w = pe_observe(wr, d_w)
            # DVE observes the bias lane via a write-once copy
            bscratch = wp.tile([128, 2], f32, name="bscratch")
            obs_b = nc.vector.tensor_copy(bscratch[:], bs[:])

            in_dmas = []      # list of lists per chunk
            out_dmas = []     # per super-chunk (final rep only)
            sc_evicts = {}    # global super-chunk -> last evict
            sc_ods = {}       # global super-chunk -> out dma
            last_mm = None
            last_evict = None
            pi = 0            # psum bank rotation
            NSC = NCHUNK // 2

            for r in range(reps):
                for c in range(NCHUNK):
                    lo = c * CHUNK - PAD
                    xt = xbufs[c]

                    observers = []
                    if r == 0:
                        chunk_dmas = []
                        if c == 0:
                            chunk_dmas.append(nc.sync.dma_start(
                                out=xt[:, :, PAD:XCOLS],
                                in_=xab[:, :, 0:CHUNK + PAD].bitcast(f32r)))
                            chunk_dmas.append(nc.sync.dma_start(
                                out=xt[:, :, 0:PAD], in_=zeros[:].bitcast(f32r)))
                        elif c == NCHUNK - 1:
                            chunk_dmas.append(nc.sync.dma_start(
                                out=xt[:, :, 0:CHUNK + PAD],
                                in_=xab[:, :, lo:T].bitcast(f32r)))
                            chunk_dmas.append(nc.sync.dma_start(
                                out=xt[:, :, CHUNK + PAD:XCOLS],
                                in_=zeros[:].bitcast(f32r)))
                        else:
                            chunk_dmas.append(nc.sync.dma_start(
                                out=xt[:], in_=xab[:, :, lo:lo + XCOLS].bitcast(f32r)))
                        in_dmas.append(chunk_dmas)

                        # PE observes this chunk's x lanes via 1-col matmuls;
                        # each observer reads only within its DMA's region.
                        for i, d in enumerate(chunk_dmas):
                            if i == 0:
                                src_ap = (xt[:, 0, PAD:PAD + 128] if c == 0
                                          else xt[:, 0, 0:128])
                            else:
                                src_ap = (xt[:, 0, 0:PAD] if c == 0
                                          else xt[:, 0, CHUNK + PAD:XCOLS])
                            observers.append(pe_observe(src_ap, d))

                    sc, half = divmod(c, 2)
                    gsc = r * NSC + sc
                    ot = obufs[gsc % NOB]
                    evict_gates = [obs_b]
                    if half == 0 and gsc >= NOB:
                        # pre-lift the recycled out buffer's history onto
                        # DVE's observed clock: one 1-wait gate per proc
                        old = gsc - NOB
                        evict_gates.append(dve_gate(sc_evicts[old]))
                        if old in sc_ods:
                            evict_gates.append(dve_gate(sc_ods[old]))

                    first_evict_of_chunk = True
                    for co in range(2):
                        for ts in range(NSUB):
                            pt = pbufs[pi % NPB]
                            pi += 1
                            first = True
                            for b in range(2):
                                for k in range(K):
                                    widx = ((k * 2 + b) * 2 + co) * 128
                                    mm = nc.tensor.matmul(
                                        pt[:],
                                        wr[:, widx:widx + 128],
                                        xt[:, b, ts * SUB + k: ts * SUB + k + SUB],
                                        start=first,
                                        stop=(b == 1 and k == K - 1),
                                    )
                                    if first:
                                        for ob in observers:
                                            add_dep_helper(
                                                mm.ins, ob.ins, sync=False,
                                                reason="order-after-observe")
                                    first = False
                                    last_mm = mm
                            off = half * CHUNK + ts * SUB
                            ev = nc.vector.tensor_scalar_add(
                                out=ot[:, co, off:off + SUB],
                                in0=pt[:],
                                scalar1=bs[:, co:co + 1],
                            )
                            if first_evict_of_chunk:
                                for g in evict_gates:
                                    add_dep_helper(ev.ins, g.ins, sync=False,
                                                   reason="order-after-gate")
                                first_evict_of_chunk = False
                            last_evict = ev

                    if half == 1:
                        sc_evicts[gsc] = last_evict
                        if r == reps - 1:
                            # SWDGE store: own output tensor + own DMASW lane
                            od = nc.gpsimd.dma_start(out=ysab[sc][:], in_=ot[:])
                            sc_ods[gsc] = od
                            out_dmas.append(od)

            # Tail flush: cover every proc with 1-dep sync nops so the final
            # drain carries at most one wait.
            tail_deps = [d for ds in in_dmas[-8:] for d in ds] + out_dmas + \
                [last_mm, last_evict]
            for dep in tail_deps:
                nop = nc.sync.nop()
                add_dep_helper(nop.ins, dep.ins, sync=True, reason="tailflush")

    return nc


def check_waits(nc):
    """Return instructions carrying more than one sync wait (walrus limit)."""
    bad = []
    for f in nc.m.functions:
        for bb in f.blocks:
            for inst in bb.instructions:
                si = inst.sync_info
                nw = len(si.on_wait) if si and si.on_wait else 0
                if nw > 1:
                    bad.append((inst.name, type(inst).__name__, nw,
                                [w.ant_name for w in si.on_wait]))
    return bad


def _pack_weights(conv_w, conv_b, lora_A, lora_B):
    w_eff = conv_w.astype(np.float32) + (
        SCALING * np.einsum(
            "or,rik->oik", lora_B.astype(np.float64),
            lora_A.astype(np.float64).reshape(RANK, CI, K))
    ).astype(np.float32)
    # wts[ci_in_block, ((k*2 + b)*2 + co)*128 + m] = w_eff[co*128+m, b*128+ci, k]
    a = w_eff.reshape(2, 128, 2, 128, K)        # [co_blk, m, ci_blk, ci, k]
    a = a.transpose(3, 4, 2, 0, 1)              # [ci, k, b, co_blk, m]
    wts = np.ascontiguousarray(a.reshape(128, K * 2 * 2 * 128), dtype=np.float32)
    bias = np.ascontiguousarray(
        conv_b.astype(np.float32).reshape(2, 128).T)  # [128, 2]
    return wts, bias


_CACHED_NC = None


def kernel(x, conv_w, conv_b, lora_A, lora_B, _trace=False):
    global _CACHED_NC
    x = np.asarray(x, dtype=np.float32)
    wts, bias = _pack_weights(np.asarray(conv_w), np.asarray(conv_b),
                              np.asarray(lora_A), np.asarray(lora_B))
    zeros = np.zeros((128, 2, PAD), dtype=np.float32)

    if _CACHED_NC is None:
        _CACHED_NC = _build_nc()
        bad = check_waits(_CACHED_NC)
        assert not bad, f"sync-wait violations: {bad[:5]}"
    nc = _CACHED_NC

    in_maps = [
        {"x": x[i], "wts": wts, "bias": bias, "zeros": zeros}
        for i in range(N_CORES)
    ]
    res = bass_utils.run_bass_kernel_spmd(
        nc, in_maps, core_ids=list(range(N_CORES)), trace=_trace)
    out = np.stack(
        [np.concatenate([res.results[i][f"y{s}"] for s in range(NCHUNK // 2)],
                        axis=1)
         for i in range(N_CORES)], axis=0)
    if _trace:
        kernel._last_exec_time_ns = res.exec_time_ns
        kernel._last_results = res
    return out


if __name__ == "__main__":
    nc = _build_nc()
    bad = check_waits(nc)
    print("violations:", bad[:10])
    n_inst = sum(len(bb.instructions) for f in nc.m.functions for bb in f.blocks)
    print("instructions:", n_inst)



# revision 19
# speedup vs baseline: 1.2175x; 1.2175x over previous
"""Conv1d (K=5, pad=2) with folded LoRA on 8 Trainium2 NeuronCores.

Strategy
--------
Data-parallel: batch 8 -> 1 batch item per core. The LoRA path is folded
into the conv weights on the host (exact up to fp32 rounding):
    W_eff = conv_w + (alpha/rank) * einsum('or,rik->oik', lora_B, lora_A)
so the device kernel is a single conv1d + bias.

Per core: y[co, t] = bias[co] + sum_{k,ci} W_eff[co, ci, k] * x[ci, t+k-2]
computed as 5 shifted matmuls accumulating in PSUM, over 2 ci-blocks and
2 co-blocks of 128. Everything rides bf16 (x, weights, and the stored y;
PSUM accumulation and the bias add stay fp32), which halves HBM traffic
vs fp32 and keeps the quantization error ~2e-3 of scale (measured) --
well under the 2e-2 gate. The host converts x to bf16 and the returned
y back to fp32.

Schedule (all targets from the TimelineSim cost model):
  - PE roofline is 640 matmuls x 512 cols @ 2.4 GHz = 136.5 us; the whole
    point of the structure below is zero PE idle between first and last
    matmul (any idle resets the 0.65->2.4 GHz pstate ramp, ~2 us each).
  - A chain of "warm" matmuls on a never-written SBUF tile keeps PE busy
    from t~0 so the ramp completes before real data arrives.
  - x is fully resident: one dedicated SBUF buffer per 1024-col chunk; the
    first chunk is split so cols [2:518) (enough for the first two PSUM
    groups) land first. Weights are packed co-major and loaded on the ACT
    HWDGE ring in parallel with x on the SP ring.
  - Each chunk has a dedicated bf16 out buffer (no recycle WAR) stored via
    SWDGE on its own DRAM tensor + DMASW lane; the last chunk is stored as
    two 512-col pieces so only ~0.25 MB trails the final matmul.
  - Conv pads are DVE memsets (no zeros tensor / extra DMAs).

Toolchain constraint baked into the structure: every instruction may carry
at most ONE sync wait (walrus setupSyncWait limit), and Tile's wait elision
is per-proc. Hence:
  - PE "observer" matmuls (2-column, scratch PSUM) absorb each x/weight DMA
    lane wait so real matmuls only wait on the DVE sem (PSUM-bank WAR /
    pad memsets -- same DVE semaphore, merged).
  - Evictions (PSUM->SBUF + bias add, fp32->bf16) run exclusively on DVE
    and wait only on PE; the bias lane is absorbed by a tiny DVE copy.
  - A tail chain of 1-dep sync nops covers all procs so the exit drain
    carries at most one wait.
"""
import sys
sys.path.insert(0, "/opt/trn_rl_repo")
import numpy as np
import ml_dtypes

from concourse import bass, mybir, tile
from concourse import bass_utils
from concourse.tile import add_dep_helper

# Problem constants (hardcoded per contract)
B = 8
CI = 256
CO = 256
K = 5
PAD = 2
T = 16384
RANK = 8
ALPHA = 16.0
SCALING = ALPHA / RANK
N_CORES = 8

# Tiling
CHUNK = 1024          # output columns per chunk
NCHUNK = T // CHUNK   # 16
SUB = 512             # matmul free dim
NSUB = CHUNK // SUB   # 2
XCOLS = CHUNK + 2 * PAD  # chunk + halo

# PE pre-warm chain: 128-col bf16 matmuls (~53-107 ns each) covering the
# DMA boot window so the pstate ramp is done when real matmuls start.
N_WARM = 36
WARM_COLS = 128


def _build_nc():
    f32 = mybir.dt.float32
    bf16 = mybir.dt.bfloat16

    nc = bass.Bass(trn_type="TRN2", debug=False)
    x = nc.dram_tensor("x", [CI, T], bf16, kind="ExternalInput").ap()
    wts = nc.dram_tensor("wts", [128, K * 2 * 2 * 128], bf16,
                         kind="ExternalInput").ap()
    bias = nc.dram_tensor("bias", [128, 2], f32, kind="ExternalInput").ap()
    # 7 SWDGE stores (one per fresh DMASW lane) + a final 256-col HWDGE store
    # on the ACT ring. Sizes shrink toward the end so only ~0.13 MB trails
    # the final matmul.
    STORE_CHUNKS = [(0, 3072), (3072, 6144), (6144, 9216), (9216, 12288),
                    (12288, 15360), (15360, 15872), (15872, 16128),
                    (16128, 16384)]
    ys = [nc.dram_tensor(f"y{i}", [CI, hi - lo], bf16,
                         kind="ExternalOutput").ap()
          for i, (lo, hi) in enumerate(STORE_CHUNKS)]

    xab = x.rearrange("(b p) t -> p b t", p=128)
    ysab = [yc.rearrange("(b p) t -> p b t", p=128) for yc in ys]

    NPB = 5   # psum accumulation banks
    WHALF = K * 2 * 128   # one co block: (b, k) stripes

    with tile.TileContext(nc) as tc:
        with tc.tile_pool(name="wp", bufs=1) as wp, \
             tc.tile_pool(name="pp", bufs=1, space="PSUM") as pp:

            # one shared scratch bank: observer columns [0:64), warm region
            # [64:320) (PSUM tiles are bank-granular -- 8 banks total)
            scratch_ps = pp.tile([128, 512], f32, name="scratch_ps",
                                 tag="scratch")
            obs_ps = scratch_ps
            warm_ps = scratch_ps
            pbufs = [pp.tile([128, SUB], f32, name=f"pt{j}", tag=f"pt{j}")
                     for j in range(NPB)]
            # dedicated banks for the last chunk's 256-col groups, used twice
            # each with the same 256-col AP (mixed-width bank reuse provokes
            # an extra conservative DVE wait on the eviction)
            tailbufs = [pp.tile([128, SUB // 2], f32, name=f"tt{j}",
                                tag=f"tt{j}") for j in range(2)]
            # x fully resident: one dedicated buffer per chunk
            xbufs = [wp.tile([128, 2, XCOLS], bf16, name=f"xt{j}", tag=f"xt{j}")
                     for j in range(NCHUNK)]
            # one out buffer per store range (no recycle; keeps each tile's
            # write-region count small so Tile's dep tracker stays exact)
            obufs = [wp.tile([128, 2, hi - lo], bf16, name=f"ot{i}",
                             tag=f"ot{i}")
                     for i, (lo, hi) in enumerate(STORE_CHUNKS)]

            # warm tile: memset by DVE at t~0, then PE chews on it to hold the
            # pstate ramp until real data arrives. Stationary is 2 cols so the
            # memset is small; out goes to scratch PSUM, never read.
            warm_sb = wp.tile([128, 2 + WARM_COLS], bf16, name="warm_sb")
            ms_warm = nc.vector.memset(warm_sb[:], 0.0)
            warm_mms = []
            for i in range(N_WARM):
                half = (i % 2) * WARM_COLS
                mm = nc.tensor.matmul(
                    warm_ps[0:2, 64 + half:64 + half + WARM_COLS],
                    warm_sb[:, 0:2],
                    warm_sb[:, 2:2 + WARM_COLS],
                    start=True, stop=True, skip_group_check=True)
                warm_mms.append(mm)

            # weights in two tiles (one per co block) so neither couples to
            # the other's DMA; ACT ring order: wA (the first groups' gate --
            # it must win the DMA FIFO race against x0b), bias, wB
            wrA = wp.tile([128, WHALF], bf16, name="wrA")
            wrB = wp.tile([128, WHALF], bf16, name="wrB")
            bs = wp.tile([128, 2], f32, name="bs")
            d_wA = nc.scalar.dma_start(out=wrA[:], in_=wts[:, 0:WHALF])
            d_b = nc.scalar.dma_start(out=bs[:], in_=bias[:])
            d_wB = nc.scalar.dma_start(out=wrB[:], in_=wts[:, WHALF:2 * WHALF])
            wrs = [wrA, wrB]

            n_obs = [0]

            def pe_observe(src_ap, dma_inst):
                """2-column matmul whose only wait is `dma_inst`'s lane.

                Reads only within the region `dma_inst` wrote; writes its own
                never-reused obs_ps column (no WAW chain)."""
                n = src_ap.shape[-1]
                m = min(128, n)
                oc = 2 * n_obs[0]
                n_obs[0] += 1
                mm = nc.tensor.matmul(obs_ps[0:m, oc:oc + 2], src_ap[:, 0:m],
                                      src_ap[:, 0:2], start=True, stop=True,
                                      skip_group_check=True)
                add_dep_helper(mm.ins, dma_inst.ins, sync=False, reason="obs-order")
                return mm

            obs_wA = pe_observe(wrA, d_wA)

            # pad memsets on DVE (chunk 0 head, chunk 15 tail) BEFORE obs_b:
            # the first real matmul waits the memset's DVE tick, and obs_b
            # (gated on the bias DMA) must not sit below it in the DVE stream
            ms_head = nc.vector.memset(xbufs[0][:, :, 0:PAD], 0.0)
            ms_tail = nc.vector.memset(
                xbufs[NCHUNK - 1][:, :, CHUNK + PAD:XCOLS], 0.0)

            # DVE observes the bias lane via a write-once copy
            bscratch = wp.tile([128, 2], f32, name="bscratch")
            obs_b = nc.vector.tensor_copy(bscratch[:], bs[:])

            # x loads on the SP HWDGE ring; chunk 0 split so the first PSUM
            # group's window lands first
            FIRST = SUB + PAD + 2  # 516 cols: covers moving windows of ts=0
            chunk_dmas = []          # per chunk: list of dmas
            d0a = nc.sync.dma_start(out=xbufs[0][:, :, PAD:PAD + FIRST],
                                    in_=xab[:, :, 0:FIRST])
            d0b = nc.sync.dma_start(out=xbufs[0][:, :, PAD + FIRST:XCOLS],
                                    in_=xab[:, :, FIRST:CHUNK + PAD])
            chunk_dmas.append([d0a, d0b])
            for c in range(1, NCHUNK):
                lo = c * CHUNK - PAD
                if c < NCHUNK - 1:
                    d = nc.sync.dma_start(out=xbufs[c][:],
                                          in_=xab[:, :, lo:lo + XCOLS])
                else:
                    d = nc.sync.dma_start(out=xbufs[c][:, :, 0:CHUNK + PAD],
                                          in_=xab[:, :, lo:T])
                chunk_dmas.append([d])

            obs_x0a = pe_observe(xbufs[0][:, 0, PAD:PAD + 128], d0a)
            obs_by_chunk = [[obs_x0a]] + [None] * (NCHUNK - 1)

            def chunk_obs(c):
                """Emit chunk c's x observer lazily, right where the PE
                stream first needs it (the scheduler keeps same-engine
                emission order, so an early observer would stall the PE)."""
                if obs_by_chunk[c] is None:
                    obs_by_chunk[c] = [
                        pe_observe(xbufs[c][:, 0, 0:128], chunk_dmas[c][0])]
                return obs_by_chunk[c]

            out_dmas = []
            last_mm = None
            last_evict = None
            pi = 0            # psum bank rotation
            si = 0            # next store index
            store_ends = [hi for _, hi in STORE_CHUNKS]

            ti = 0            # dedicated tail bank index

            def emit_group(c, co, off, width, extra_obs=()):
                """One PSUM accumulation group: 10 matmuls + 1 eviction.
                off/width are chunk-local output columns."""
                nonlocal pi, ti, last_mm, last_evict
                if width == SUB:
                    pt = pbufs[pi % NPB]
                    pi += 1
                else:
                    pt = tailbufs[ti % 2]
                    ti += 1
                xt = xbufs[c]
                first = True
                for b in range(2):
                    for k in range(K):
                        widx = (b * K + k) * 128
                        mm = nc.tensor.matmul(
                            pt[:, 0:width],
                            wrs[co][:, widx:widx + 128],
                            xt[:, b, off + k: off + k + width],
                            start=first,
                            stop=(b == 1 and k == K - 1),
                        )
                        if first:
                            for ob in (*chunk_obs(c), *extra_obs):
                                add_dep_helper(mm.ins, ob.ins, sync=False,
                                               reason="order-after-observe")
                        first = False
                        last_mm = mm
                goff = c * CHUNK + off
                s = next(i for i, (lo, hi) in enumerate(STORE_CHUNKS)
                         if lo <= goff < hi)
                slo = STORE_CHUNKS[s][0]
                ev = nc.vector.tensor_scalar_add(
                    out=obufs[s][:, co, goff - slo:goff - slo + width],
                    in0=pt[:, 0:width],
                    scalar1=bs[:, co:co + 1],
                )
                if c == 0 and co == 0 and off == 0:
                    add_dep_helper(ev.ins, obs_b.ins, sync=False,
                                   reason="order-after-gate")
                last_evict = ev
                return goff + width

            def flush_stores(done_cols, both_co):
                nonlocal si
                if not both_co:
                    return
                while si < len(STORE_CHUNKS) and done_cols >= store_ends[si]:
                    od = nc.gpsimd.dma_start(out=ysab[si][:],
                                             in_=obufs[si][:])
                    out_dmas.append(od)
                    si += 1

            obs_x0b = [None]
            for c in range(NCHUNK - 1):
                # co-outer: co=0 only needs wrA, giving wrB's DMA extra slack
                for co in range(2):
                    for ts in range(NSUB):
                        extra = ()
                        if c == 0 and co == 0 and ts == 0:
                            extra = (obs_wA,)
                        elif c == 0 and ts == 1:
                            if obs_x0b[0] is None:
                                obs_x0b[0] = pe_observe(
                                    xbufs[0][:, 0,
                                             PAD + FIRST:PAD + FIRST + 128],
                                    d0b)
                            extra = (obs_x0b[0],)
                        done = emit_group(c, co, ts * SUB, SUB, extra)
                        flush_stores(done, co == 1)

            # last chunk: ts-outer with a 512 + 256 + 256 column split so the
            # final store is tiny and leaves right after the last eviction
            c = NCHUNK - 1
            for off, width in [(0, SUB), (SUB, SUB // 2),
                               (SUB + SUB // 2, SUB // 2)]:
                for co in range(2):
                    done = emit_group(c, co, off, width)
                    flush_stores(done, co == 1)

            # Tail flush: cover every proc with 1-dep sync nops so the final
            # drain carries at most one wait. Every DMA lane\'s LAST user needs
            # a nop; order by expected completion so the final nop is cheap.
            tail_deps = [d for ds in chunk_dmas[-8:] for d in ds] + \
                [last_mm, last_evict] + out_dmas
            for dep in tail_deps:
                nop = nc.sync.nop()
                add_dep_helper(nop.ins, dep.ins, sync=True, reason="tailflush")

    return nc


def check_waits(nc):
    """Return instructions carrying more than one sync wait (walrus limit)."""
    bad = []
    for f in nc.m.functions:
        for bb in f.blocks:
            for inst in bb.instructions:
                si = inst.sync_info
                nw = len(si.on_wait) if si and si.on_wait else 0
                if nw > 1:
                    bad.append((inst.name, type(inst).__name__, nw,
                                [w.ant_name for w in si.on_wait]))
    return bad


def _pack_weights(conv_w, conv_b, lora_A, lora_B):
    w_eff = conv_w.astype(np.float32) + (
        SCALING * np.einsum(
            "or,rik->oik", lora_B.astype(np.float64),
            lora_A.astype(np.float64).reshape(RANK, CI, K))
    ).astype(np.float32)
    # wts[ci, ((co*2 + b)*K + k)*128 + m] = w_eff[co*128+m, b*128+ci, k]
    a = w_eff.reshape(2, 128, 2, 128, K)        # [co_blk, m, ci_blk, ci, k]
    a = a.transpose(3, 0, 2, 4, 1)              # [ci, co_blk, b, k, m]
    wts = np.ascontiguousarray(
        a.reshape(128, 2 * 2 * K * 128)).astype(ml_dtypes.bfloat16)
    bias = np.ascontiguousarray(
        conv_b.astype(np.float32).reshape(2, 128).T)  # [128, 2]
    return wts, bias


_CACHED_NC = None


def kernel(x, conv_w, conv_b, lora_A, lora_B, _trace=False):
    global _CACHED_NC
    x = np.asarray(x, dtype=np.float32).astype(ml_dtypes.bfloat16)
    wts, bias = _pack_weights(np.asarray(conv_w), np.asarray(conv_b),
                              np.asarray(lora_A), np.asarray(lora_B))

    if _CACHED_NC is None:
        _CACHED_NC = _build_nc()
        bad = check_waits(_CACHED_NC)
        assert not bad, f"sync-wait violations: {bad[:5]}"
    nc = _CACHED_NC

    in_maps = [
        {"x": x[i], "wts": wts, "bias": bias}
        for i in range(N_CORES)
    ]
    res = bass_utils.run_bass_kernel_spmd(
        nc, in_maps, core_ids=list(range(N_CORES)), trace=_trace)
    out = np.stack(
        [np.concatenate(
            [np.asarray(res.results[i][f"y{s}"]).astype(np.float32)
             for s in range(8)], axis=1)
         for i in range(N_CORES)], axis=0)
    if _trace:
        kernel._last_exec_time_ns = res.exec_time_ns
        kernel._last_results = res
    return out


if __name__ == "__main__":
    nc = _build_nc()
    bad = check_waits(nc)
    print("violations:", bad[:10])
    n_inst = sum(len(bb.instructions) for f in nc.m.functions for bb in f.blocks)
    print("instructions:", n_inst)


# revision 31
# speedup vs baseline: 1.2256x; 1.0067x over previous
"""Conv1d (K=5, pad=2) with folded LoRA on 8 Trainium2 NeuronCores.

Strategy
--------
Data-parallel: batch 8 -> 1 batch item per core. The LoRA path is folded
into the conv weights on the host (exact up to fp32 rounding):
    W_eff = conv_w + (alpha/rank) * einsum('or,rik->oik', lora_B, lora_A)
so the device kernel is a single conv1d + bias.

Per core: y[co, t] = bias[co] + sum_{k,ci} W_eff[co, ci, k] * x[ci, t+k-2]
computed as 5 shifted matmuls accumulating in PSUM, over 2 ci-blocks and
2 co-blocks of 128. Everything rides bf16 (x, weights, and the stored y;
PSUM accumulation and the bias add stay fp32), which halves HBM traffic
vs fp32 and keeps the quantization error ~2e-3 of scale (measured) --
well under the 2e-2 gate. The host converts x to bf16 and the returned
y back to fp32.

Schedule (all targets from the TimelineSim cost model):
  - PE roofline is 640 matmuls x 512 cols @ 2.4 GHz = 136.5 us; the whole
    point of the structure below is zero PE idle between first and last
    matmul (any idle resets the 0.65->2.4 GHz pstate ramp, ~2 us each).
  - A chain of "warm" matmuls on a never-written SBUF tile keeps PE busy
    from t~0 so the ramp completes before real data arrives.
  - x is fully resident: one dedicated SBUF buffer per 1024-col chunk; the
    first chunk is split so cols [2:518) (enough for the first two PSUM
    groups) land first. Weights are packed co-major and loaded on the ACT
    HWDGE ring in parallel with x on the SP ring.
  - Each chunk has a dedicated bf16 out buffer (no recycle WAR) stored via
    SWDGE on its own DRAM tensor + DMASW lane; the last chunk is stored as
    two 512-col pieces so only ~0.25 MB trails the final matmul.
  - Conv pads are DVE memsets (no zeros tensor / extra DMAs).

Toolchain constraint baked into the structure: every instruction may carry
at most ONE sync wait (walrus setupSyncWait limit), and Tile's wait elision
is per-proc. Hence:
  - PE "observer" matmuls (2-column, scratch PSUM) absorb each x/weight DMA
    lane wait so real matmuls only wait on the DVE sem (PSUM-bank WAR /
    pad memsets -- same DVE semaphore, merged).
  - Evictions (PSUM->SBUF + bias add, fp32->bf16) run exclusively on DVE
    and wait only on PE; the bias lane is absorbed by a tiny DVE copy.
  - A tail chain of 1-dep sync nops covers all procs so the exit drain
    carries at most one wait.
"""
import sys
sys.path.insert(0, "/opt/trn_rl_repo")
import numpy as np
import ml_dtypes

from concourse import bass, mybir, tile
from concourse import bass_utils
from concourse.tile import add_dep_helper

# Problem constants (hardcoded per contract)
B = 8
CI = 256
CO = 256
K = 5
PAD = 2
T = 16384
RANK = 8
ALPHA = 16.0
SCALING = ALPHA / RANK
N_CORES = 8

# Tiling
CHUNK = 1024          # output columns per chunk
NCHUNK = T // CHUNK   # 16
SUB = 512             # matmul free dim
NSUB = CHUNK // SUB   # 2
XCOLS = CHUNK + 2 * PAD  # chunk + halo

# PE pre-warm chain covering the DMA boot window so the pstate ramp is done
# when real matmuls start: 128-col bf16 matmuls (~107 ns at mid pstate) with
# a 64-col tail for fine-grained landing right on the first real matmul.
N_WARM_128 = 24
N_WARM_64 = 14
WARM_COLS = 128


def _build_nc():
    f32 = mybir.dt.float32
    bf16 = mybir.dt.bfloat16

    nc = bass.Bass(trn_type="TRN2", debug=False)
    x = nc.dram_tensor("x", [CI, T], bf16, kind="ExternalInput").ap()
    wts = nc.dram_tensor("wts", [128, K * 2 * 2 * 128], bf16,
                         kind="ExternalInput").ap()
    bias = nc.dram_tensor("bias", [128, 2], f32, kind="ExternalInput").ap()
    # 7 SWDGE stores (one per fresh DMASW lane) + a final 256-col HWDGE store
    # on the ACT ring. Sizes shrink toward the end so only ~0.13 MB trails
    # the final matmul.
    STORE_CHUNKS = [(0, 3072), (3072, 6144), (6144, 9216), (9216, 12288),
                    (12288, 15360), (15360, 15872), (15872, 16128),
                    (16128, 16384)]
    ys = [nc.dram_tensor(f"y{i}", [CI, hi - lo], bf16,
                         kind="ExternalOutput").ap()
          for i, (lo, hi) in enumerate(STORE_CHUNKS)]

    xab = x.rearrange("(b p) t -> p b t", p=128)
    ysab = [yc.rearrange("(b p) t -> p b t", p=128) for yc in ys]

    NPB = 5   # psum accumulation banks
    WHALF = K * 2 * 128   # one co block: (b, k) stripes

    with tile.TileContext(nc) as tc:
        with tc.tile_pool(name="wp", bufs=1) as wp, \
             tc.tile_pool(name="pp", bufs=1, space="PSUM") as pp:

            # one shared scratch bank: observer columns [0:64), warm region
            # [64:320) (PSUM tiles are bank-granular -- 8 banks total)
            scratch_ps = pp.tile([128, 512], f32, name="scratch_ps",
                                 tag="scratch")
            obs_ps = scratch_ps
            warm_ps = scratch_ps
            pbufs = [pp.tile([128, SUB], f32, name=f"pt{j}", tag=f"pt{j}")
                     for j in range(NPB)]
            # dedicated banks for the last chunk's 256-col groups, alternated
            # so consecutive groups never share a bank (and never reused at a
            # different width -- that provokes an extra conservative DVE wait)
            tailbufs = [pp.tile([128, SUB // 2], f32, name="tt0", tag="tt0"),
                        pp.tile([128, SUB // 2], f32, name="tt1", tag="tt1")]
            # x fully resident in three tiles: chunk 0, chunks 1-3, chunks
            # 4-15 (few, large DMAs: each HWDGE DMA costs ~630 ns of ring
            # setup, and total HWDGE count must stay at 7 so the final store
            # is the 8th -- owning a fresh lane, hence a single sync wait)
            XT1_CH = 3
            xt0 = wp.tile([128, 2, XCOLS], bf16, name="xt0", tag="xt0")
            xt1 = wp.tile([128, 2, XT1_CH * CHUNK + 2 * PAD], bf16,
                          name="xt1", tag="xt1")
            xt2 = wp.tile([128, 2, (NCHUNK - 1 - XT1_CH) * CHUNK + 2 * PAD],
                          bf16, name="xt2", tag="xt2")

            def xtile(c):
                """(tile, column offset of chunk c's window base) pairs."""
                if c == 0:
                    return xt0, 0
                if c <= XT1_CH:
                    return xt1, (c - 1) * CHUNK
                return xt2, (c - 1 - XT1_CH) * CHUNK
            # one out buffer per store range (no recycle; keeps each tile's
            # write-region count small so Tile's dep tracker stays exact)
            obufs = [wp.tile([128, 2, hi - lo], bf16, name=f"ot{i}",
                             tag=f"ot{i}")
                     for i, (lo, hi) in enumerate(STORE_CHUNKS)]

            # warm tile: memset by DVE at t~0, then PE chews on it to hold the
            # pstate ramp until real data arrives. Stationary is 2 cols so the
            # memset is small; out goes to scratch PSUM, never read.
            warm_sb = wp.tile([128, 2 + WARM_COLS], bf16, name="warm_sb")
            ms_warm = nc.gpsimd.memset(warm_sb[:], 0.0)
            warm_mms = []
            for i in range(N_WARM_128 + N_WARM_64):
                cols = WARM_COLS if i < N_WARM_128 else 64
                half = (i % 2) * WARM_COLS
                mm = nc.tensor.matmul(
                    warm_ps[0:2, 64 + half:64 + half + cols],
                    warm_sb[:, 0:2],
                    warm_sb[:, 2:2 + cols],
                    start=True, stop=True, skip_group_check=True)
                warm_mms.append(mm)

            # weights in two tiles (one per co block) so neither couples to
            # the other's DMA. wA is further split into b-stripes so the
            # first group's b=0 matmuls can start after only 640 weight cols;
            # bias rides SWDGE (fast request pipeline, and the DMASW lanes
            # hold exactly bias + 7 stores; the 8th store is HWDGE).
            wrA = wp.tile([128, WHALF], bf16, name="wrA")
            wrB = wp.tile([128, WHALF], bf16, name="wrB")
            bs = wp.tile([128, 2], f32, name="bs")
            WQ = WHALF // 2
            d_b = nc.gpsimd.dma_start(out=bs[:], in_=bias[:])
            d_wA1 = nc.scalar.dma_start(out=wrA[:, 0:WQ], in_=wts[:, 0:WQ])
            d_wA2 = nc.scalar.dma_start(out=wrA[:, WQ:WHALF],
                                        in_=wts[:, WQ:WHALF])
            d_wB = nc.scalar.dma_start(out=wrB[:], in_=wts[:, WHALF:2 * WHALF])
            wrs = [wrA, wrB]

            n_obs = [0]

            def pe_observe(src_ap, dma_inst):
                """2-column matmul whose only wait is `dma_inst`'s lane.

                Reads only within the region `dma_inst` wrote; writes its own
                never-reused obs_ps column (no WAW chain)."""
                n = src_ap.shape[-1]
                m = min(128, n)
                oc = 2 * n_obs[0]
                n_obs[0] += 1
                mm = nc.tensor.matmul(obs_ps[0:m, oc:oc + 2], src_ap[:, 0:m],
                                      src_ap[:, 0:2], start=True, stop=True,
                                      skip_group_check=True)
                add_dep_helper(mm.ins, dma_inst.ins, sync=False, reason="obs-order")
                return mm

            obs_wA1 = pe_observe(wrA[:, 0:WQ], d_wA1)

            # pad memsets on DVE (chunk 0 head, chunk 15 tail) BEFORE obs_b:
            # the first real matmul waits the memset's DVE tick, and obs_b
            # (gated on the bias DMA) must not sit below it in the DVE stream
            XT2_W = (NCHUNK - 1 - XT1_CH) * CHUNK + 2 * PAD
            ms_head = nc.vector.memset(xt0[:, :, 0:PAD], 0.0)
            ms_tail = nc.vector.memset(xt2[:, :, XT2_W - PAD:XT2_W], 0.0)

            # DVE observes the bias lane via a write-once copy
            bscratch = wp.tile([128, 2], f32, name="bscratch")
            obs_b = nc.vector.tensor_copy(bscratch[:], bs[:])

            # chunk-0 pieces on the SP ring; the two big tiles ride the ACT
            # ring AFTER the weights (a big transfer requested before wB
            # would starve the co=1 groups)
            FIRST = SUB + PAD + 2  # 516 cols: covers moving windows of ts=0
            d0a = nc.sync.dma_start(out=xt0[:, :, PAD:PAD + FIRST],
                                    in_=xab[:, :, 0:FIRST])
            d0b = nc.sync.dma_start(out=xt0[:, :, PAD + FIRST:XCOLS],
                                    in_=xab[:, :, FIRST:CHUNK + PAD])
            d_x1 = nc.scalar.dma_start(
                out=xt1[:], in_=xab[:, :, CHUNK - PAD:(XT1_CH + 1) * CHUNK + PAD])
            d_x2 = nc.scalar.dma_start(
                out=xt2[:, :, 0:XT2_W - PAD],
                in_=xab[:, :, (XT1_CH + 1) * CHUNK - PAD:T])
            chunk_dmas = [[d0a, d0b], [d_x1], [d_x2]]

            obs_x0a = pe_observe(xt0[:, 0, PAD:PAD + 128], d0a)
            obs_by_tile = {0: [obs_x0a], 1: None, 2: None}
            tile_dma = {1: d_x1, 2: d_x2}
            tile_buf = {1: xt1, 2: xt2}

            def chunk_obs(c):
                """Emit the x-tile observer lazily, right where the PE stream
                first needs it (the scheduler keeps same-engine emission
                order, so an early observer would stall the PE)."""
                t = 0 if c == 0 else (1 if c <= XT1_CH else 2)
                if obs_by_tile[t] is None:
                    obs_by_tile[t] = [pe_observe(
                        tile_buf[t][:, 0, 0:128], tile_dma[t])]
                return obs_by_tile[t]

            out_dmas = []
            last_mm = None
            last_evict = None
            pi = 0            # psum bank rotation
            si = 0            # next store index
            store_ends = [hi for _, hi in STORE_CHUNKS]

            ti = 0            # dedicated tail bank index

            def emit_half(c, co, ts, b, pt, start, extra_obs=()):
                """Five matmuls: one (b, k*) half of a PSUM group."""
                nonlocal last_mm
                xt, xoff = xtile(c)
                for k in range(K):
                    widx = (b * K + k) * 128
                    mm = nc.tensor.matmul(
                        pt[:],
                        wrs[co][:, widx:widx + 128],
                        xt[:, b, xoff + ts * SUB + k: xoff + ts * SUB + k + SUB],
                        start=start and k == 0,
                        stop=(b == 1 and k == K - 1),
                        skip_group_check=True,
                    )
                    if k == 0:
                        for ob in extra_obs:
                            add_dep_helper(mm.ins, ob.ins, sync=False,
                                           reason="order-after-observe")
                    last_mm = mm

            def emit_group(c, co, off, width, extra_obs=()):
                """One PSUM accumulation group: 10 matmuls + 1 eviction.
                off/width are chunk-local output columns."""
                nonlocal pi, ti, last_mm, last_evict
                if width == SUB:
                    pt = pbufs[pi % NPB]
                    pi += 1
                else:
                    pt = tailbufs[ti % 2]
                    ti += 1
                xt, xoff = xtile(c)
                first = True
                for b in range(2):
                    for k in range(K):
                        widx = (b * K + k) * 128
                        mm = nc.tensor.matmul(
                            pt[:, 0:width],
                            wrs[co][:, widx:widx + 128],
                            xt[:, b, xoff + off + k: xoff + off + k + width],
                            start=first,
                            stop=(b == 1 and k == K - 1),
                        )
                        if first:
                            obs = extra_obs if (c == 0 and extra_obs) \
                                else (*chunk_obs(c), *extra_obs)
                            for ob in obs:
                                add_dep_helper(mm.ins, ob.ins, sync=False,
                                               reason="order-after-observe")
                        first = False
                        last_mm = mm
                goff = c * CHUNK + off
                s = next(i for i, (lo, hi) in enumerate(STORE_CHUNKS)
                         if lo <= goff < hi)
                slo = STORE_CHUNKS[s][0]
                ev = nc.vector.tensor_scalar_add(
                    out=obufs[s][:, co, goff - slo:goff - slo + width],
                    in0=pt[:, 0:width],
                    scalar1=bs[:, co:co + 1],
                )
                last_evict = ev
                return goff + width

            def flush_stores(done_cols, both_co):
                nonlocal si
                if not both_co:
                    return
                while si < len(STORE_CHUNKS) and done_cols >= store_ends[si]:
                    if si == len(STORE_CHUNKS) - 1:
                        od = nc.sync.dma_start(out=ysab[si][:],
                                               in_=obufs[si][:])
                    else:
                        od = nc.gpsimd.dma_start(out=ysab[si][:],
                                                 in_=obufs[si][:])
                    out_dmas.append(od)
                    si += 1

            # chunk 0, co=0, ts=0 is the first real group; its b=1 half's
            # first matmul carries the wA2 stripe's DMA wait directly (its
            # PSUM bank is fresh and xt0 is already observed, so it is the
            # sole sync wait)
            first_evict = [None]

            def first_gate(ev):
                if first_evict[0] is None:
                    first_evict[0] = ev
                    add_dep_helper(ev.ins, obs_b.ins, sync=False,
                                   reason="order-after-gate")

            obs_x0b = [None]
            for c in range(NCHUNK - 1):
                # co-outer: co=0 only needs wrA, giving wrB's DMA extra slack
                for co in range(2):
                    for ts in range(NSUB):
                        extra = ()
                        if c == 0 and co == 0 and ts == 0:
                            extra = (obs_x0a, obs_wA1)
                        elif c == 0 and ts == 1:
                            if obs_x0b[0] is None:
                                obs_x0b[0] = pe_observe(
                                    xt0[:, 0,
                                        PAD + FIRST:PAD + FIRST + 128], d0b)
                            extra = (obs_x0b[0],)
                        done = emit_group(c, co, ts * SUB, SUB, extra)
                        first_gate(last_evict)
                        flush_stores(done, co == 1)

            # last chunk: ts-outer with a 512 + 256 + 256 column split so the
            # final store is tiny and leaves right after the last eviction
            c = NCHUNK - 1
            for off, width in [(0, SUB), (SUB, SUB // 2),
                               (SUB + SUB // 2, SUB // 2)]:
                for co in range(2):
                    done = emit_group(c, co, off, width)
                    flush_stores(done, co == 1)

            # Tail flush: cover every proc with 1-dep sync nops so the final
            # drain carries at most one wait. Every DMA lane\'s LAST user needs
            # a nop; order by expected completion so the final nop is cheap.
            tail_deps = [d_b, d_wA1, d_wA2, d_wB] + \
                [d for ds in chunk_dmas for d in ds] + \
                [last_mm, last_evict] + out_dmas
            for dep in tail_deps:
                nop = nc.sync.nop()
                add_dep_helper(nop.ins, dep.ins, sync=True, reason="tailflush")

    return nc


def check_waits(nc):
    """Return instructions carrying more than one sync wait (walrus limit)."""
    bad = []
    for f in nc.m.functions:
        for bb in f.blocks:
            for inst in bb.instructions:
                si = inst.sync_info
                nw = len(si.on_wait) if si and si.on_wait else 0
                if nw > 1:
                    bad.append((inst.name, type(inst).__name__, nw,
                                [w.ant_name for w in si.on_wait]))
    return bad


def _pack_weights(conv_w, conv_b, lora_A, lora_B):
    w_eff = conv_w.astype(np.float32) + (
        SCALING * np.einsum(
            "or,rik->oik", lora_B.astype(np.float64),
            lora_A.astype(np.float64).reshape(RANK, CI, K))
    ).astype(np.float32)
    # wts[ci, ((co*2 + b)*K + k)*128 + m] = w_eff[co*128+m, b*128+ci, k]
    a = w_eff.reshape(2, 128, 2, 128, K)        # [co_blk, m, ci_blk, ci, k]
    a = a.transpose(3, 0, 2, 4, 1)              # [ci, co_blk, b, k, m]
    wts = np.ascontiguousarray(
        a.reshape(128, 2 * 2 * K * 128)).astype(ml_dtypes.bfloat16)
    bias = np.ascontiguousarray(
        conv_b.astype(np.float32).reshape(2, 128).T)  # [128, 2]
    return wts, bias


_CACHED_NC = None


def kernel(x, conv_w, conv_b, lora_A, lora_B, _trace=False):
    global _CACHED_NC
    x = np.asarray(x, dtype=np.float32).astype(ml_dtypes.bfloat16)
    wts, bias = _pack_weights(np.asarray(conv_w), np.asarray(conv_b),
                              np.asarray(lora_A), np.asarray(lora_B))

    if _CACHED_NC is None:
        _CACHED_NC = _build_nc()
        bad = check_waits(_CACHED_NC)
        assert not bad, f"sync-wait violations: {bad[:5]}"
    nc = _CACHED_NC

    in_maps = [
        {"x": x[i], "wts": wts, "bias": bias}
        for i in range(N_CORES)
    ]
    res = bass_utils.run_bass_kernel_spmd(
        nc, in_maps, core_ids=list(range(N_CORES)), trace=_trace)
    out = np.stack(
        [np.concatenate(
            [np.asarray(res.results[i][f"y{s}"]).astype(np.float32)
             for s in range(8)], axis=1)
         for i in range(N_CORES)], axis=0)
    if _trace:
        kernel._last_exec_time_ns = res.exec_time_ns
        kernel._last_results = res
    return out


if __name__ == "__main__":
    nc = _build_nc()
    bad = check_waits(nc)
    print("violations:", bad[:10])
    n_inst = sum(len(bb.instructions) for f in nc.m.functions for bb in f.blocks)
    print("instructions:", n_inst)
